# revision 34
# baseline (speedup 1.0000x reference)
"""CrossAttention kernel for 8 TRN2 NeuronCores (Bass/Tile).

Reference computation (per batch b):
    q = x @ Wq ; k = ctx @ Wk ; v = ctx @ Wv        (heads H=8, dh=64)
    attn = softmax(q k^T / sqrt(dh)) ; o = attn @ v
    out = o @ Wo + bo

Sharding (8 cores): core c -> (batch b = c//2, head-group hg = c%2).
Each core handles 4 heads of one batch over the full sequence; the two
head-group partial outputs per batch are summed on the host (Wo is
sliced by rows, so partials add exactly).

Layout strategy: everything on-chip is kept "feature-major" (transposed)
so no on-chip transpose is ever needed:
  - host passes xT=[512,2048], cT=[512,2048] (bf16), pre-packed into the
    SBUF-resident layout and chunked along the sequence so the DMA lands
    as one 4KB-contiguous descriptor per partition per chunk
  - QT = Wq^T x^T, KT = Wk^T c^T  (d on partitions, seq on free)
  - V  = c @ Wv natural            (seq on partitions, d on free)
  - S^T = K_h Q_h^T                (keys m on partitions, queries n free)
  - E = exp(S^T/8)  (ScalarE, read PSUM directly; logits are tiny so no
    max-subtraction is needed -- softmax is shift-invariant)
  - O' = [V_h | ones]-stationary matmul: one matmul yields O'^T rows
    0..63 AND the softmax denominators replicated on rows 64..127
  - O^T normalized with 1/sums, concatenated; Y^T = Wo_s^T O^T emitted
    per query chunk so the output projection and store overlap the
    ScalarE-bound attention instead of trailing it

Timeline notes (from the 200us baseline trace): ScalarE exp is the
bottleneck engine (142us busy); the wins here are head latency (input
DMA chunked on the two HWDGE queues; first exp at ~13us instead of
37us) and tail latency (Y projection + output DMA interleaved per
chunk instead of a 24us serial tail).
"""

import os

import ml_dtypes
import numpy as np

import concourse.bass as bass
import concourse.mybir as mybir
import concourse.tile as tile
from concourse import bacc
from concourse.bass_utils import run_bass_kernel_spmd

BF16 = mybir.dt.bfloat16
F32 = mybir.dt.float32

D = 512          # model dim
N = 2048         # query seq len
M = 2048         # key seq len
HPC = 4          # heads per core
DH = 64          # head dim
DS = HPC * DH    # per-core inner dim = 256
SCALE = 1.0 / 8.0  # 1/sqrt(64)
P = 128
KT_D = D // P    # 4 k-tiles over model dim
MT = M // P      # 16 m-tiles over keys
NCHUNK = 4       # input-sequence DMA chunks (512 cols each)
CW = N // NCHUNK  # 512

_NBF = ml_dtypes.bfloat16


def _build_nc():
    nc = bacc.Bacc(None, target_bir_lowering=False)

    # activations pre-packed on host: [chunk, p, ko, 512]
    xT = nc.declare_dram_parameter("xT", [NCHUNK, P, KT_D, CW], BF16, isOutput=False)
    cT = nc.declare_dram_parameter("cT", [NCHUNK, P, KT_D, CW], BF16, isOutput=False)
    # weights pre-packed on host: [p, ko, d]
    wq = nc.declare_dram_parameter("wq", [P, KT_D, DS], BF16, isOutput=False)
    wk = nc.declare_dram_parameter("wk", [P, KT_D, DS], BF16, isOutput=False)
    wv = nc.declare_dram_parameter("wv", [P, KT_D, DS], BF16, isOutput=False)
    wo = nc.declare_dram_parameter("wo", [P, DS // P, D], BF16, isOutput=False)
    # chunk-major so each per-chunk store is one contiguous 4KB descriptor
    # per partition (strided layouts measured at only ~35 GB/s)
    yT = nc.declare_dram_parameter("yT", [N // 512, P, D // P, 512], BF16,
                                   isOutput=True)

    with tile.TileContext(nc) as tc:
        _emit(tc, xT, cT, wq, wk, wv, wo, yT)
    nc.finalize()
    return nc


def _emit(tc, xT, cT, wq, wk, wv, wo, yT):
    nc = tc.nc
    NCH = 1024           # n-chunk for the attention inner loop
    EXP = mybir.ActivationFunctionType.Exp

    from contextlib import ExitStack

    with ExitStack() as ctx:
        const = ctx.enter_context(tc.tile_pool(name="const", bufs=1))
        work = ctx.enter_context(tc.tile_pool(name="work", bufs=6))
        yout = ctx.enter_context(tc.tile_pool(name="yout", bufs=2))
        ps_s = ctx.enter_context(tc.tile_pool(name="ps_s", bufs=2, space="PSUM"))
        ps_o = ctx.enter_context(tc.tile_pool(name="ps_o", bufs=2, space="PSUM"))
        ps_p = ctx.enter_context(tc.tile_pool(name="ps_p", bufs=2, space="PSUM"))

        # ---- resident SBUF tensors ----
        # activations chunk-major so each input-chunk DMA writes one
        # contiguous 4KB run per partition (strided dst APs fragment the
        # transfer into 1KB descriptors and halve DMA bandwidth)
        xT_sb = const.tile([P, NCHUNK, KT_D, CW], BF16)
        cT_sb = const.tile([P, NCHUNK, KT_D, CW], BF16)
        wq_sb = const.tile([P, KT_D, DS], BF16)
        wk_sb = const.tile([P, KT_D, DS], BF16)
        wv_sb = const.tile([P, KT_D, DS], BF16)
        wo_sb = const.tile([P, DS // P, D], BF16)
        QT_sb = const.tile([P, DS // P, N], BF16)
        KT_sb = const.tile([P, DS // P, M], BF16)
        # per (m-tile, head): 128 stationary columns = [V_h (64) | ones (64)]
        # so one matmul yields O'^T rows 0..63 AND the softmax sums
        # replicated on rows 64..127 (pre-broadcast for the normalize).
        Vp_sb = const.tile([P, MT, HPC, P], BF16)
        Ocat = const.tile([P, DS // P, N], BF16)
        tbl = const.tile([1, 2], F32)

        # ---- input DMA: all on the sync HWDGE queue. A single dma_start
        # already stripes across all 16 SDMA engines (full HBM bandwidth),
        # and one queue keeps arrival strictly in first-use order. Issuing
        # from nc.scalar would be a mistake: DMA-issue instructions that hit
        # a semaphore-reuse wait would stall the ScalarE stream and the
        # ACTIVATEs behind it.
        nc.sync.dma_start(wk_sb[:], wk[:])
        nc.sync.dma_start(wq_sb[:], wq[:])
        nc.sync.dma_start(cT_sb[:, 0], cT[0])
        nc.sync.dma_start(xT_sb[:, 0], xT[0])
        nc.sync.dma_start(wv_sb[:], wv[:])
        for c in range(1, NCHUNK):
            nc.sync.dma_start(cT_sb[:, c], cT[c])
            nc.sync.dma_start(xT_sb[:, c], xT[c])
        nc.sync.dma_start(wo_sb[:], wo[:])

        # tiny dummy exp so the ~2.7us ACT_TABLE_LOAD overlaps the DMA wait
        nc.vector.memset(tbl[:, 0:1], 0.0)
        nc.scalar.activation(tbl[:, 1:2], tbl[:, 0:1], EXP)
        # warm-up matmuls on a small dedicated tile (NOT the big ones-memset
        # below, which takes ~4us on DVE and would delay them): the PE HAM
        # clock gate needs ~3.4us of sustained activity to lift the PE from
        # 1.2 to 2.4 GHz; burn that during the DMA wait so the first
        # projection chain runs at full clock.
        warm = const.tile([P, 512], BF16)
        nc.vector.memset(warm[:], 1.0)
        wps = ps_p.tile([P, 512], F32, tag="psp", name="warm")
        for _ in range(12):
            nc.tensor.matmul(
                wps[0:DH, :512],
                lhsT=warm[:, 0:DH],
                rhs=warm[:],
                start=True,
                stop=True,
            )
        # ones columns for the denominator trick; DVE is idle during the DMA
        nc.vector.memset(Vp_sb[:, :, :, DH:P], 1.0)

        # ---- just-in-time projections (emitted inside the attention loop
        # so PE reaches the first softmax chunk as soon as its DMA chunk
        # lands instead of draining all projection matmuls first) ----
        proj_done = set()

        def emit_qk(w_sb, src_sb, dst_sb, dt, ch, key):
            if (key, dt, ch) in proj_done:
                return
            proj_done.add((key, dt, ch))
            ps = ps_p.tile([P, 512], F32, tag="psp", name=f"{key}{dt}{ch}")
            for kt in range(KT_D):
                nc.tensor.matmul(
                    ps[:, :512],
                    lhsT=w_sb[:, kt, dt * P:(dt + 1) * P],
                    rhs=src_sb[:, ch, kt, :],
                    start=(kt == 0),
                    stop=(kt == KT_D - 1),
                )
            nc.vector.tensor_copy(
                dst_sb[:, dt, ch * 512:(ch + 1) * 512], ps[:, :512]
            )

        def emit_v(mt):
            if ("v", mt) in proj_done:
                return
            proj_done.add(("v", mt))
            ps = ps_p.tile([P, 512], F32, tag="psp", name=f"v{mt}")
            for kt in range(KT_D):
                nc.tensor.matmul(
                    ps[:, :DS],
                    lhsT=cT_sb[:, mt // 4, kt, (mt % 4) * P:(mt % 4 + 1) * P],
                    rhs=wv_sb[:, kt, :],
                    start=(kt == 0),
                    stop=(kt == KT_D - 1),
                )
            nc.vector.tensor_copy(
                Vp_sb[:, mt, :, 0:DH],
                ps[:, 0:DS].rearrange("p (h d) -> p h d", h=HPC),
            )

        # ---- Y^T = Wo_s^T O^T for one query chunk; emitted piecewise as
        # filler work inside the NEXT chunk's inner loop so it fills PE
        # slack during the ScalarE-bound attention instead of blocking the
        # in-order PE queue at chunk boundaries.
        NS = 512  # n sub-chunk
        ytiles = {}

        def emit_y(nch, dt4):
            n0 = nch * NS
            if dt4 == 0:
                ytiles[nch] = yout.tile(
                    [P, D // P, NS], BF16, tag="y", name=f"yt{nch}"
                )
            yt = ytiles[nch]
            ps = ps_p.tile([P, NS], F32, tag="psp", name=f"y{nch}{dt4}")
            for kt in range(DS // P):
                nc.tensor.matmul(
                    ps[:, :NS],
                    lhsT=wo_sb[:, kt, dt4 * P:(dt4 + 1) * P],
                    rhs=Ocat[:, kt, n0:n0 + NS],
                    start=(kt == 0),
                    stop=(kt == DS // P - 1),
                )
            nc.vector.tensor_copy(yt[:, dt4, :], ps[:, :NS])
            if dt4 == D // P - 1:
                nc.sync.dma_start(yT[nch], yt[:])

        # head pairs (2p, 2p+1) live at partition offsets 0/64 of d-tile p:
        # their K=64 S-matmuls use disjoint PE row groups (concurrent), and
        # share one [128, 1024] PSUM tile -> a single 1024-wide exp.
        # One flat software pipeline over all 8 (nch, pr) blocks x 16 m-tiles:
        # AV lags S/exp by LAG stages GLOBALLY (across block boundaries), so
        # a stalled AV (waiting on a po PSUM slot freed by the previous
        # block's normalize on DVE) never blocks the S matmuls that feed
        # ScalarE, and ScalarE enters each new block with a full runway.
        LAG = 4
        NB = (N // NS) * (HPC // 2)   # 8 blocks
        TOT = NB * MT                 # 128 stages
        ebufs = [None] * (LAG + 2)
        po_blk = {}
        # ScalarE exp is the bottleneck engine (~141us busy); offload these
        # (block, mt) tiles to VectorE via the Schraudolph trick in bf16
        # space: exp(x) ~= bitcast_bf16(int16(x * 2^7/ln2 + (127*2^7 - C)))
        # one fused mult+add tensor_scalar with int16 convert-on-write, then
        # the AV matmul streams the tile as bf16 like the ACT path. ~1.7%
        # rms weight error on the offloaded key tiles; final output error
        # scales as 1.7% * sqrt(fraction offloaded).
        SCHR = {(g, mt) for g in range(2, NB) for mt in (6, 11)}
        EXA = SCALE * (1 << 7) / float(np.log(2.0))
        EXB = 127.0 * (1 << 7) - 366393.0 / (1 << 16)

        def fillers(nch, pr, mt):
            # background work with already-satisfied deps, spread over the
            # loop: pr-1 projections during block 0, the previous chunk's
            # output projection, the next chunk's Q projection
            if nch == 0 and pr == 0:
                if mt in (5, 7, 9, 11):
                    emit_qk(wk_sb, cT_sb, KT_sb, 1, (mt - 5) // 2, "k")
                if mt == 13:
                    emit_qk(wq_sb, xT_sb, QT_sb, 1, 0, "q")
            if pr == 0 and nch > 0 and mt in (4, 7, 10, 13):
                emit_y(nch - 1, (mt - 4) // 3)
            if pr == 1 and nch + 1 < N // NS and mt in (5, 9):
                emit_qk(wq_sb, xT_sb, QT_sb, (mt - 5) // 4, nch + 1, "q")

        for s in range(TOT + LAG):
            if s < TOT:
                g, mt = divmod(s, MT)
                nch, pr = divmod(g, HPC // 2)
                n0 = nch * NS
                if mt == 0:
                    po_blk[g] = [
                        ps_o.tile([P, NS], F32, tag="po", name=f"po{g}{i}")
                        for i in range(2)
                    ]
                emit_qk(wk_sb, cT_sb, KT_sb, pr, mt // 4, "k")
                emit_qk(wq_sb, xT_sb, QT_sb, pr, n0 // 512, "q")
                fillers(nch, pr, mt)
                st = ps_s.tile([P, NCH], F32, tag="ps")
                for i in range(2):
                    dp = i * DH
                    nc.tensor.matmul(
                        st[:, i * NS:(i + 1) * NS],
                        lhsT=KT_sb[dp:dp + DH, pr, mt * P:(mt + 1) * P],
                        rhs=QT_sb[dp:dp + DH, pr, n0:n0 + NS],
                        start=True,
                        stop=True,
                    )
                if (g, mt) in SCHR:
                    ei = work.tile([P, NCH], mybir.dt.int16, tag="ei",
                                   name=f"ei{g}")
                    nc.vector.tensor_scalar(
                        ei[:], st[:], EXA, EXB,
                        mybir.AluOpType.mult, mybir.AluOpType.add,
                    )
                    ebufs[s % len(ebufs)] = ei
                else:
                    e = work.tile([P, NCH], BF16, tag="e")
                    nc.scalar.activation(e[:], st[:], EXP, scale=SCALE)
                    ebufs[s % len(ebufs)] = e
            if s >= LAG:
                sa = s - LAG
                g, ma = divmod(sa, MT)
                nch, pr = divmod(g, HPC // 2)
                n0 = nch * NS
                emit_v(ma)
                e = ebufs[sa % len(ebufs)]
                schr = (g, ma) in SCHR
                for i, h in enumerate((2 * pr, 2 * pr + 1)):
                    rhs = e[:, i * NS:(i + 1) * NS]
                    if schr:
                        rhs = rhs.bitcast(BF16)
                    nc.tensor.matmul(
                        po_blk[g][i][:],
                        lhsT=Vp_sb[:, ma, h, :],
                        rhs=rhs,
                        start=(ma == 0),
                        stop=(ma == MT - 1),
                    )
                if ma == MT - 1:
                    # normalize: O^T = O'^T * (1/sums); sums already sit on
                    # rows 64..127 of the po accumulators
                    for i in range(2):
                        dp = i * DH
                        sc = work.tile([DH, NS], F32, tag="sc")
                        nc.vector.tensor_copy(sc[:], po_blk[g][i][DH:P, :])
                        rc = work.tile([DH, NS], F32, tag="rc")
                        nc.vector.reciprocal_approx_fast(rc[:], sc[:])
                        nc.vector.tensor_tensor(
                            Ocat[dp:dp + DH, pr, n0:n0 + NS],
                            po_blk[g][i][0:DH, :],
                            rc[:],
                            mybir.AluOpType.mult,
                        )
                    del po_blk[g]

        # last chunk's output projection runs in the tail
        for dt4 in range(D // P):
            emit_y(N // NS - 1, dt4)


def _install_ntff_hook():
    """Best-effort NTFF profiling under axon: provide the antenv.axon_hooks
    shim the boot code looks for, and avoid the artifact upload."""
    try:
        import sys
        import types

        import concourse.bass_utils as bu

        bu.upload_artifacts = lambda d: d  # no S3 in this sandbox
        try:
            from antenv.axon_hooks import get_axon_ntff_profile_hook  # noqa: F401
            return  # already present
        except ImportError:
            pass
        import antenv
        from trn_agent_boot.trn_boot import _ntff_profile_via_ctypes

        mod = types.ModuleType("antenv.axon_hooks")
        _state = {"hook": _ntff_profile_via_ctypes("/opt/axon/libaxon_pjrt.so")}
        mod.set_axon_ntff_profile_hook = lambda h: _state.__setitem__("hook", h)
        mod.get_axon_ntff_profile_hook = lambda: _state["hook"]
        sys.modules["antenv.axon_hooks"] = mod
        antenv.axon_hooks = mod
    except Exception as e:  # pragma: no cover
        print(f"ntff hook install failed ({e}); running without trace")


def _pack_act(a):
    """[D, N] -> [chunk, p, ko, 512] so each chunk DMA is one contiguous
    4KB descriptor per partition."""
    return np.ascontiguousarray(
        a.reshape(KT_D, P, NCHUNK, CW).transpose(2, 1, 0, 3)
    ).astype(_NBF)


def _pack_w(w):
    """[D, DS] -> [p, ko, d]."""
    ds = w.shape[1]
    return np.ascontiguousarray(
        w.reshape(KT_D, P, ds).transpose(1, 0, 2)
    ).astype(_NBF)


def kernel(x, context, Wq, Wk, Wv, Wo, bo):
    x = np.asarray(x, dtype=np.float32)
    context = np.asarray(context, dtype=np.float32)
    Wq = np.asarray(Wq, dtype=np.float32)
    Wk = np.asarray(Wk, dtype=np.float32)
    Wv = np.asarray(Wv, dtype=np.float32)
    Wo = np.asarray(Wo, dtype=np.float32)
    bo = np.asarray(bo, dtype=np.float32)
    B = x.shape[0]

    in_maps = []
    for c in range(8):
        b, hg = c // 2, c % 2
        sl = slice(hg * DS, (hg + 1) * DS)
        wos = Wo[sl, :]  # [DS, D]
        in_maps.append({
            "xT": _pack_act(x[b].T),
            "cT": _pack_act(context[b].T),
            "wq": _pack_w(Wq[:, sl]),
            "wk": _pack_w(Wk[:, sl]),
            "wv": _pack_w(Wv[:, sl]),
            "wo": np.ascontiguousarray(
                wos.reshape(DS // P, P, D).transpose(1, 0, 2)
            ).astype(_NBF),
        })

    nc = _build_nc()
    trace = bool(int(os.environ.get("BASS_KERNEL_TRACE", "0")))
    if trace:
        _install_ntff_hook()
    res = run_bass_kernel_spmd(nc, in_maps, list(range(8)), trace=trace)
    if trace and res.exec_time_ns is not None:
        print(f"HW exec time: {res.exec_time_ns} ns")

    out = np.empty((B, N, D), dtype=np.float32)
    for b in range(B):
        # yT is [nch, p, dt, n'] bf16; d = dt*128 + p, n = nch*512 + n'
        y0 = res.results[2 * b]["yT"].astype(np.float32)
        y1 = res.results[2 * b + 1]["yT"].astype(np.float32)
        yt = (y0 + y1).transpose(2, 1, 0, 3).reshape(D, N)
        out[b] = yt.T + bo[None, :]
    return out


# revision 35
# speedup vs baseline: 1.0035x; 1.0035x over previous
"""CrossAttention kernel for 8 TRN2 NeuronCores (Bass/Tile).

Reference computation (per batch b):
    q = x @ Wq ; k = ctx @ Wk ; v = ctx @ Wv        (heads H=8, dh=64)
    attn = softmax(q k^T / sqrt(dh)) ; o = attn @ v
    out = o @ Wo + bo

Sharding (8 cores): core c -> (batch b = c//2, head-group hg = c%2).
Each core handles 4 heads of one batch over the full sequence; the two
head-group partial outputs per batch are summed on the host (Wo is
sliced by rows, so partials add exactly).

Layout strategy: everything on-chip is kept "feature-major" (transposed)
so no on-chip transpose is ever needed:
  - host passes xT=[512,2048], cT=[512,2048] (bf16), pre-packed into the
    SBUF-resident layout and chunked along the sequence so the DMA lands
    as one 4KB-contiguous descriptor per partition per chunk
  - QT = Wq^T x^T, KT = Wk^T c^T  (d on partitions, seq on free)
  - V  = c @ Wv natural            (seq on partitions, d on free)
  - S^T = K_h Q_h^T                (keys m on partitions, queries n free)
  - E = exp(S^T/8)  (ScalarE, read PSUM directly; logits are tiny so no
    max-subtraction is needed -- softmax is shift-invariant)
  - O' = [V_h | ones]-stationary matmul: one matmul yields O'^T rows
    0..63 AND the softmax denominators replicated on rows 64..127
  - O^T normalized with 1/sums, concatenated; Y^T = Wo_s^T O^T emitted
    per query chunk so the output projection and store overlap the
    ScalarE-bound attention instead of trailing it

Performance structure (from perfetto traces; 200us baseline -> ~178us):
  - ScalarE exp (16.7M elems/core at 1 elem/cycle/lane) and TensorE
    column streaming (the attention matrix passes through the PE twice:
    S writes + AV reads = 262K PSUM columns at ~1 col/cycle, plus
    ~57K projection columns) are BOTH near 130-140us busy; the kernel
    runs as one flat software pipeline (S/exp with AV lagging LAG
    stages globally) so neither engine ever head-of-line blocks the
    other at block boundaries.
  - ~12 of 128 exp tiles are offloaded from ScalarE to VectorE via a
    bf16-space Schraudolph approximation to rebalance the two engines.
  - Head: input DMA on one HWDGE queue in first-use order with
    contiguous-per-partition chunk layouts (~320GB/s), PE warm-up
    matmuls during the DMA wait (HAM clock gate), early ACT table load.
  - Tail: per-chunk output projection is emitted as filler work inside
    the next chunk's loop; chunk-major yT layout keeps the final store
    at ~190GB/s (strided layout measured 35GB/s).
"""

import os

import ml_dtypes
import numpy as np

import concourse.bass as bass
import concourse.mybir as mybir
import concourse.tile as tile
from concourse import bacc
from concourse.bass_utils import run_bass_kernel_spmd

BF16 = mybir.dt.bfloat16
F32 = mybir.dt.float32

D = 512          # model dim
N = 2048         # query seq len
M = 2048         # key seq len
HPC = 4          # heads per core
DH = 64          # head dim
DS = HPC * DH    # per-core inner dim = 256
SCALE = 1.0 / 8.0  # 1/sqrt(64)
P = 128
KT_D = D // P    # 4 k-tiles over model dim
MT = M // P      # 16 m-tiles over keys
NCHUNK = 4       # input-sequence DMA chunks (512 cols each)
CW = N // NCHUNK  # 512

_NBF = ml_dtypes.bfloat16


def _build_nc():
    nc = bacc.Bacc(None, target_bir_lowering=False)

    # activations pre-packed on host: [chunk, p, ko, 512]
    xT = nc.declare_dram_parameter("xT", [NCHUNK, P, KT_D, CW], BF16, isOutput=False)
    cT = nc.declare_dram_parameter("cT", [NCHUNK, P, KT_D, CW], BF16, isOutput=False)
    # weights pre-packed on host: [p, ko, d]
    wq = nc.declare_dram_parameter("wq", [P, KT_D, DS], BF16, isOutput=False)
    wk = nc.declare_dram_parameter("wk", [P, KT_D, DS], BF16, isOutput=False)
    wv = nc.declare_dram_parameter("wv", [P, KT_D, DS], BF16, isOutput=False)
    wo = nc.declare_dram_parameter("wo", [P, DS // P, D], BF16, isOutput=False)
    # chunk-major so each per-chunk store is one contiguous 4KB descriptor
    # per partition (strided layouts measured at only ~35 GB/s)
    yT = nc.declare_dram_parameter("yT", [N // 512, P, D // P, 512], BF16,
                                   isOutput=True)

    with tile.TileContext(nc) as tc:
        _emit(tc, xT, cT, wq, wk, wv, wo, yT)
    nc.finalize()
    return nc


def _emit(tc, xT, cT, wq, wk, wv, wo, yT):
    nc = tc.nc
    NCH = 1024           # n-chunk for the attention inner loop
    EXP = mybir.ActivationFunctionType.Exp

    from contextlib import ExitStack

    with ExitStack() as ctx:
        const = ctx.enter_context(tc.tile_pool(name="const", bufs=1))
        work = ctx.enter_context(tc.tile_pool(name="work", bufs=6))
        yout = ctx.enter_context(tc.tile_pool(name="yout", bufs=2))
        ps_s = ctx.enter_context(tc.tile_pool(name="ps_s", bufs=2, space="PSUM"))
        ps_o = ctx.enter_context(tc.tile_pool(name="ps_o", bufs=2, space="PSUM"))
        ps_p = ctx.enter_context(tc.tile_pool(name="ps_p", bufs=2, space="PSUM"))

        # ---- resident SBUF tensors ----
        # activations chunk-major so each input-chunk DMA writes one
        # contiguous 4KB run per partition (strided dst APs fragment the
        # transfer into 1KB descriptors and halve DMA bandwidth)
        xT_sb = const.tile([P, NCHUNK, KT_D, CW], BF16)
        cT_sb = const.tile([P, NCHUNK, KT_D, CW], BF16)
        wq_sb = const.tile([P, KT_D, DS], BF16)
        wk_sb = const.tile([P, KT_D, DS], BF16)
        wv_sb = const.tile([P, KT_D, DS], BF16)
        wo_sb = const.tile([P, DS // P, D], BF16)
        QT_sb = const.tile([P, DS // P, N], BF16)
        KT_sb = const.tile([P, DS // P, M], BF16)
        # per (m-tile, head): 128 stationary columns = [V_h (64) | ones (64)]
        # so one matmul yields O'^T rows 0..63 AND the softmax sums
        # replicated on rows 64..127 (pre-broadcast for the normalize).
        Vp_sb = const.tile([P, MT, HPC, P], BF16)
        Ocat = const.tile([P, DS // P, N], BF16)
        tbl = const.tile([1, 2], F32)

        # ---- input DMA: all on the sync HWDGE queue. A single dma_start
        # already stripes across all 16 SDMA engines (full HBM bandwidth),
        # and one queue keeps arrival strictly in first-use order. Issuing
        # from nc.scalar would be a mistake: DMA-issue instructions that hit
        # a semaphore-reuse wait would stall the ScalarE stream and the
        # ACTIVATEs behind it.
        nc.sync.dma_start(wk_sb[:], wk[:])
        nc.sync.dma_start(wq_sb[:], wq[:])
        nc.sync.dma_start(cT_sb[:, 0], cT[0])
        nc.sync.dma_start(xT_sb[:, 0], xT[0])
        nc.sync.dma_start(wv_sb[:], wv[:])
        for c in range(1, NCHUNK):
            nc.sync.dma_start(cT_sb[:, c], cT[c])
            nc.sync.dma_start(xT_sb[:, c], xT[c])
        nc.sync.dma_start(wo_sb[:], wo[:])

        # tiny dummy exp so the ~2.7us ACT_TABLE_LOAD overlaps the DMA wait
        nc.vector.memset(tbl[:, 0:1], 0.0)
        nc.scalar.activation(tbl[:, 1:2], tbl[:, 0:1], EXP)
        # warm-up matmuls on a small dedicated tile (NOT the big ones-memset
        # below, which takes ~4us on DVE and would delay them): the PE HAM
        # clock gate needs ~3.4us of sustained activity to lift the PE from
        # 1.2 to 2.4 GHz; burn that during the DMA wait so the first
        # projection chain runs at full clock.
        warm = const.tile([P, 512], BF16)
        nc.vector.memset(warm[:], 1.0)
        wps = ps_p.tile([P, 512], F32, tag="psp", name="warm")
        for _ in range(12):
            nc.tensor.matmul(
                wps[0:DH, :512],
                lhsT=warm[:, 0:DH],
                rhs=warm[:],
                start=True,
                stop=True,
            )
        # ones columns for the denominator trick; DVE is idle during the DMA
        nc.vector.memset(Vp_sb[:, :, :, DH:P], 1.0)

        # ---- just-in-time projections (emitted inside the attention loop
        # so PE reaches the first softmax chunk as soon as its DMA chunk
        # lands instead of draining all projection matmuls first) ----
        proj_done = set()

        def emit_qk(w_sb, src_sb, dst_sb, dt, ch, key):
            if (key, dt, ch) in proj_done:
                return
            proj_done.add((key, dt, ch))
            ps = ps_p.tile([P, 512], F32, tag="psp", name=f"{key}{dt}{ch}")
            for kt in range(KT_D):
                nc.tensor.matmul(
                    ps[:, :512],
                    lhsT=w_sb[:, kt, dt * P:(dt + 1) * P],
                    rhs=src_sb[:, ch, kt, :],
                    start=(kt == 0),
                    stop=(kt == KT_D - 1),
                )
            nc.vector.tensor_copy(
                dst_sb[:, dt, ch * 512:(ch + 1) * 512], ps[:, :512]
            )

        def emit_v(mt):
            if ("v", mt) in proj_done:
                return
            proj_done.add(("v", mt))
            ps = ps_p.tile([P, 512], F32, tag="psp", name=f"v{mt}")
            for kt in range(KT_D):
                nc.tensor.matmul(
                    ps[:, :DS],
                    lhsT=cT_sb[:, mt // 4, kt, (mt % 4) * P:(mt % 4 + 1) * P],
                    rhs=wv_sb[:, kt, :],
                    start=(kt == 0),
                    stop=(kt == KT_D - 1),
                )
            nc.vector.tensor_copy(
                Vp_sb[:, mt, :, 0:DH],
                ps[:, 0:DS].rearrange("p (h d) -> p h d", h=HPC),
            )

        # ---- Y^T = Wo_s^T O^T for one query chunk; emitted piecewise as
        # filler work inside the NEXT chunk's inner loop so it fills PE
        # slack during the ScalarE-bound attention instead of blocking the
        # in-order PE queue at chunk boundaries.
        NS = 512  # n sub-chunk
        ytiles = {}

        def emit_y(nch, dt4):
            n0 = nch * NS
            if dt4 == 0:
                ytiles[nch] = yout.tile(
                    [P, D // P, NS], BF16, tag="y", name=f"yt{nch}"
                )
            yt = ytiles[nch]
            ps = ps_p.tile([P, NS], F32, tag="psp", name=f"y{nch}{dt4}")
            for kt in range(DS // P):
                nc.tensor.matmul(
                    ps[:, :NS],
                    lhsT=wo_sb[:, kt, dt4 * P:(dt4 + 1) * P],
                    rhs=Ocat[:, kt, n0:n0 + NS],
                    start=(kt == 0),
                    stop=(kt == DS // P - 1),
                )
            nc.vector.tensor_copy(yt[:, dt4, :], ps[:, :NS])
            if dt4 == D // P - 1:
                nc.sync.dma_start(yT[nch], yt[:])

        # head pairs (2p, 2p+1) live at partition offsets 0/64 of d-tile p:
        # their K=64 S-matmuls use disjoint PE row groups (concurrent), and
        # share one [128, 1024] PSUM tile -> a single 1024-wide exp.
        # One flat software pipeline over all 8 (nch, pr) blocks x 16 m-tiles:
        # AV lags S/exp by LAG stages GLOBALLY (across block boundaries), so
        # a stalled AV (waiting on a po PSUM slot freed by the previous
        # block's normalize on DVE) never blocks the S matmuls that feed
        # ScalarE, and ScalarE enters each new block with a full runway.
        LAG = 4
        NB = (N // NS) * (HPC // 2)   # 8 blocks
        TOT = NB * MT                 # 128 stages
        ebufs = [None] * (LAG + 2)
        po_blk = {}
        # ScalarE exp is the bottleneck engine (~141us busy); offload these
        # (block, mt) tiles to VectorE via the Schraudolph trick in bf16
        # space: exp(x) ~= bitcast_bf16(int16(x * 2^7/ln2 + (127*2^7 - C)))
        # one fused mult+add tensor_scalar with int16 convert-on-write, then
        # the AV matmul streams the tile as bf16 like the ACT path. ~1.7%
        # rms weight error on the offloaded key tiles; final output error
        # scales as 1.7% * sqrt(fraction offloaded).
        SCHR = {(g, mt) for g in range(2, NB) for mt in (6, 11)}
        EXA = SCALE * (1 << 7) / float(np.log(2.0))
        EXB = 127.0 * (1 << 7) - 366393.0 / (1 << 16)

        def fillers(nch, pr, mt):
            # background work with already-satisfied deps, spread over the
            # loop: pr-1 projections during block 0, the previous chunk's
            # output projection, the next chunk's Q projection
            if nch == 0 and pr == 0:
                if mt in (5, 7, 9, 11):
                    emit_qk(wk_sb, cT_sb, KT_sb, 1, (mt - 5) // 2, "k")
                if mt == 13:
                    emit_qk(wq_sb, xT_sb, QT_sb, 1, 0, "q")
            if pr == 0 and nch > 0 and mt in (4, 7, 10, 13):
                emit_y(nch - 1, (mt - 4) // 3)
            if pr == 1 and nch + 1 < N // NS and mt in (5, 9):
                emit_qk(wq_sb, xT_sb, QT_sb, (mt - 5) // 4, nch + 1, "q")

        for s in range(TOT + LAG):
            if s < TOT:
                g, mt = divmod(s, MT)
                nch, pr = divmod(g, HPC // 2)
                n0 = nch * NS
                if mt == 0:
                    po_blk[g] = [
                        ps_o.tile([P, NS], F32, tag="po", name=f"po{g}{i}")
                        for i in range(2)
                    ]
                emit_qk(wk_sb, cT_sb, KT_sb, pr, mt // 4, "k")
                emit_qk(wq_sb, xT_sb, QT_sb, pr, n0 // 512, "q")
                fillers(nch, pr, mt)
                st = ps_s.tile([P, NCH], F32, tag="ps")
                for i in range(2):
                    dp = i * DH
                    nc.tensor.matmul(
                        st[:, i * NS:(i + 1) * NS],
                        lhsT=KT_sb[dp:dp + DH, pr, mt * P:(mt + 1) * P],
                        rhs=QT_sb[dp:dp + DH, pr, n0:n0 + NS],
                        start=True,
                        stop=True,
                    )
                if (g, mt) in SCHR:
                    ei = work.tile([P, NCH], mybir.dt.int16, tag="ei",
                                   name=f"ei{g}")
                    nc.vector.tensor_scalar(
                        ei[:], st[:], EXA, EXB,
                        mybir.AluOpType.mult, mybir.AluOpType.add,
                    )
                    ebufs[s % len(ebufs)] = ei
                else:
                    e = work.tile([P, NCH], BF16, tag="e")
                    nc.scalar.activation(e[:], st[:], EXP, scale=SCALE)
                    ebufs[s % len(ebufs)] = e
            if s >= LAG:
                sa = s - LAG
                g, ma = divmod(sa, MT)
                nch, pr = divmod(g, HPC // 2)
                n0 = nch * NS
                emit_v(ma)
                e = ebufs[sa % len(ebufs)]
                schr = (g, ma) in SCHR
                for i, h in enumerate((2 * pr, 2 * pr + 1)):
                    rhs = e[:, i * NS:(i + 1) * NS]
                    if schr:
                        rhs = rhs.bitcast(BF16)
                    nc.tensor.matmul(
                        po_blk[g][i][:],
                        lhsT=Vp_sb[:, ma, h, :],
                        rhs=rhs,
                        start=(ma == 0),
                        stop=(ma == MT - 1),
                    )
                if ma == MT - 1:
                    # normalize: O^T = O'^T * (1/sums); sums already sit on
                    # rows 64..127 of the po accumulators
                    for i in range(2):
                        dp = i * DH
                        sc = work.tile([DH, NS], F32, tag="sc")
                        nc.vector.tensor_copy(sc[:], po_blk[g][i][DH:P, :])
                        rc = work.tile([DH, NS], F32, tag="rc")
                        nc.vector.reciprocal_approx_fast(rc[:], sc[:])
                        nc.vector.tensor_tensor(
                            Ocat[dp:dp + DH, pr, n0:n0 + NS],
                            po_blk[g][i][0:DH, :],
                            rc[:],
                            mybir.AluOpType.mult,
                        )
                    del po_blk[g]

        # last chunk's output projection runs in the tail
        for dt4 in range(D // P):
            emit_y(N // NS - 1, dt4)


def _install_ntff_hook():
    """Best-effort NTFF profiling under axon: provide the antenv.axon_hooks
    shim the boot code looks for, and avoid the artifact upload."""
    try:
        import sys
        import types

        import concourse.bass_utils as bu

        bu.upload_artifacts = lambda d: d  # no S3 in this sandbox
        try:
            from antenv.axon_hooks import get_axon_ntff_profile_hook  # noqa: F401
            return  # already present
        except ImportError:
            pass
        import antenv
        from trn_agent_boot.trn_boot import _ntff_profile_via_ctypes

        mod = types.ModuleType("antenv.axon_hooks")
        _state = {"hook": _ntff_profile_via_ctypes("/opt/axon/libaxon_pjrt.so")}
        mod.set_axon_ntff_profile_hook = lambda h: _state.__setitem__("hook", h)
        mod.get_axon_ntff_profile_hook = lambda: _state["hook"]
        sys.modules["antenv.axon_hooks"] = mod
        antenv.axon_hooks = mod
    except Exception as e:  # pragma: no cover
        print(f"ntff hook install failed ({e}); running without trace")


def _pack_act(a):
    """[D, N] -> [chunk, p, ko, 512] so each chunk DMA is one contiguous
    4KB descriptor per partition."""
    return np.ascontiguousarray(
        a.reshape(KT_D, P, NCHUNK, CW).transpose(2, 1, 0, 3)
    ).astype(_NBF)


def _pack_w(w):
    """[D, DS] -> [p, ko, d]."""
    ds = w.shape[1]
    return np.ascontiguousarray(
        w.reshape(KT_D, P, ds).transpose(1, 0, 2)
    ).astype(_NBF)


def kernel(x, context, Wq, Wk, Wv, Wo, bo):
    x = np.asarray(x, dtype=np.float32)
    context = np.asarray(context, dtype=np.float32)
    Wq = np.asarray(Wq, dtype=np.float32)
    Wk = np.asarray(Wk, dtype=np.float32)
    Wv = np.asarray(Wv, dtype=np.float32)
    Wo = np.asarray(Wo, dtype=np.float32)
    bo = np.asarray(bo, dtype=np.float32)
    B = x.shape[0]

    in_maps = []
    for c in range(8):
        b, hg = c // 2, c % 2
        sl = slice(hg * DS, (hg + 1) * DS)
        wos = Wo[sl, :]  # [DS, D]
        in_maps.append({
            "xT": _pack_act(x[b].T),
            "cT": _pack_act(context[b].T),
            "wq": _pack_w(Wq[:, sl]),
            "wk": _pack_w(Wk[:, sl]),
            "wv": _pack_w(Wv[:, sl]),
            "wo": np.ascontiguousarray(
                wos.reshape(DS // P, P, D).transpose(1, 0, 2)
            ).astype(_NBF),
        })

    nc = _build_nc()
    trace = bool(int(os.environ.get("BASS_KERNEL_TRACE", "0")))
    if trace:
        _install_ntff_hook()
    res = run_bass_kernel_spmd(nc, in_maps, list(range(8)), trace=trace)
    if trace and res.exec_time_ns is not None:
        print(f"HW exec time: {res.exec_time_ns} ns")

    out = np.empty((B, N, D), dtype=np.float32)
    for b in range(B):
        # yT is [nch, p, dt, n'] bf16; d = dt*128 + p, n = nch*512 + n'
        y0 = res.results[2 * b]["yT"].astype(np.float32)
        y1 = res.results[2 * b + 1]["yT"].astype(np.float32)
        yt = (y0 + y1).transpose(2, 1, 0, 3).reshape(D, N)
        out[b] = yt.T + bo[None, :]
    return out


# revision 38
# speedup vs baseline: 1.0322x; 1.0286x over previous
"""CrossAttention kernel for 8 TRN2 NeuronCores (Bass/Tile).

Reference computation (per batch b):
    q = x @ Wq ; k = ctx @ Wk ; v = ctx @ Wv        (heads H=8, dh=64)
    attn = softmax(q k^T / sqrt(dh)) ; o = attn @ v
    out = o @ Wo + bo

Sharding (8 cores): core c -> (batch b = c//2, head-group hg = c%2).
Each core handles 4 heads of one batch over the full sequence; the two
head-group partial outputs per batch are summed on the host (Wo is
sliced by rows, so partials add exactly).

Layout strategy: everything on-chip is kept "feature-major" (transposed)
so no on-chip transpose is ever needed:
  - host passes xT=[512,2048], cT=[512,2048] (bf16), pre-packed into the
    SBUF-resident layout and chunked along the sequence so the DMA lands
    as one 4KB-contiguous descriptor per partition per chunk
  - QT = Wq^T x^T, KT = Wk^T c^T  (d on partitions, seq on free)
  - V  = c @ Wv natural            (seq on partitions, d on free)
  - S^T = K_h Q_h^T                (keys m on partitions, queries n free)
  - E = exp(S^T/8)  (ScalarE, read PSUM directly; logits are tiny so no
    max-subtraction is needed -- softmax is shift-invariant)
  - O' = [V_h | ones]-stationary matmul: one matmul yields O'^T rows
    0..63 AND the softmax denominators replicated on rows 64..127
  - O^T normalized with 1/sums, concatenated; Y^T = Wo_s^T O^T emitted
    per query chunk so the output projection and store overlap the
    ScalarE-bound attention instead of trailing it

Performance structure (from perfetto traces; 200us baseline -> ~178us):
  - ScalarE exp (16.7M elems/core at 1 elem/cycle/lane) and TensorE
    column streaming (the attention matrix passes through the PE twice:
    S writes + AV reads = 262K PSUM columns at ~1 col/cycle, plus
    ~57K projection columns) are BOTH near 130-140us busy; the kernel
    runs as one flat software pipeline (S/exp with AV lagging LAG
    stages globally) so neither engine ever head-of-line blocks the
    other at block boundaries.
  - ~12 of 128 exp tiles are offloaded from ScalarE to VectorE via a
    bf16-space Schraudolph approximation to rebalance the two engines.
  - Head: input DMA on one HWDGE queue in first-use order with
    contiguous-per-partition chunk layouts (~320GB/s), PE warm-up
    matmuls during the DMA wait (HAM clock gate), early ACT table load.
  - Tail: per-chunk output projection is emitted as filler work inside
    the next chunk's loop; chunk-major yT layout keeps the final store
    at ~190GB/s (strided layout measured 35GB/s).
"""

import os

import ml_dtypes
import numpy as np

import concourse.bass as bass
import concourse.mybir as mybir
import concourse.tile as tile
from concourse import bacc
from concourse.bass_utils import run_bass_kernel_spmd

BF16 = mybir.dt.bfloat16
F32 = mybir.dt.float32

D = 512          # model dim
N = 2048         # query seq len
M = 2048         # key seq len
HPC = 4          # heads per core
DH = 64          # head dim
DS = HPC * DH    # per-core inner dim = 256
SCALE = 1.0 / 8.0  # 1/sqrt(64)
P = 128
KT_D = D // P    # 4 k-tiles over model dim
MT = M // P      # 16 m-tiles over keys
NCHUNK = 4       # input-sequence DMA chunks (512 cols each)
CW = N // NCHUNK  # 512

_NBF = ml_dtypes.bfloat16


def _build_nc():
    nc = bacc.Bacc(None, target_bir_lowering=False)

    # activations pre-packed on host: [chunk, p, ko, 512]
    xT = nc.declare_dram_parameter("xT", [NCHUNK, P, KT_D, CW], BF16, isOutput=False)
    cT = nc.declare_dram_parameter("cT", [NCHUNK, P, KT_D, CW], BF16, isOutput=False)
    # weights pre-packed on host: [p, ko, d]
    wq = nc.declare_dram_parameter("wq", [P, KT_D, DS], BF16, isOutput=False)
    wk = nc.declare_dram_parameter("wk", [P, KT_D, DS], BF16, isOutput=False)
    wv = nc.declare_dram_parameter("wv", [P, KT_D, DS], BF16, isOutput=False)
    wo = nc.declare_dram_parameter("wo", [P, DS // P, D], BF16, isOutput=False)
    # chunk-major so each per-chunk store is one contiguous 4KB descriptor
    # per partition (strided layouts measured at only ~35 GB/s)
    yT = nc.declare_dram_parameter("yT", [N // 512, P, D // P, 512], BF16,
                                   isOutput=True)

    with tile.TileContext(nc) as tc:
        _emit(tc, xT, cT, wq, wk, wv, wo, yT)
    nc.finalize()
    return nc


def _emit(tc, xT, cT, wq, wk, wv, wo, yT):
    nc = tc.nc
    NCH = 1024           # n-chunk for the attention inner loop
    EXP = mybir.ActivationFunctionType.Exp

    from contextlib import ExitStack

    with ExitStack() as ctx:
        const = ctx.enter_context(tc.tile_pool(name="const", bufs=1))
        work = ctx.enter_context(tc.tile_pool(name="work", bufs=8))
        yout = ctx.enter_context(tc.tile_pool(name="yout", bufs=2))
        ps_s = ctx.enter_context(tc.tile_pool(name="ps_s", bufs=2, space="PSUM"))
        ps_o = ctx.enter_context(tc.tile_pool(name="ps_o", bufs=2, space="PSUM"))
        ps_p = ctx.enter_context(tc.tile_pool(name="ps_p", bufs=2, space="PSUM"))

        # ---- resident SBUF tensors ----
        # activations chunk-major so each input-chunk DMA writes one
        # contiguous 4KB run per partition (strided dst APs fragment the
        # transfer into 1KB descriptors and halve DMA bandwidth)
        xT_sb = const.tile([P, NCHUNK, KT_D, CW], BF16)
        cT_sb = const.tile([P, NCHUNK, KT_D, CW], BF16)
        wq_sb = const.tile([P, KT_D, DS], BF16)
        wk_sb = const.tile([P, KT_D, DS], BF16)
        wv_sb = const.tile([P, KT_D, DS], BF16)
        wo_sb = const.tile([P, DS // P, D], BF16)
        QT_sb = const.tile([P, DS // P, N], BF16)
        KT_sb = const.tile([P, DS // P, M], BF16)
        # per (m-tile, head): 128 stationary columns = [V_h (64) | ones (64)]
        # so one matmul yields O'^T rows 0..63 AND the softmax sums
        # replicated on rows 64..127 (pre-broadcast for the normalize).
        Vp_sb = const.tile([P, MT, HPC, P], BF16)
        Ocat = const.tile([P, DS // P, N], BF16)
        tbl = const.tile([1, 2], F32)

        # ---- input DMA: all on the sync HWDGE queue. A single dma_start
        # already stripes across all 16 SDMA engines (full HBM bandwidth),
        # and one queue keeps arrival strictly in first-use order. Issuing
        # from nc.scalar would be a mistake: DMA-issue instructions that hit
        # a semaphore-reuse wait would stall the ScalarE stream and the
        # ACTIVATEs behind it.
        nc.sync.dma_start(wk_sb[:], wk[:])
        nc.sync.dma_start(wq_sb[:], wq[:])
        nc.sync.dma_start(cT_sb[:, 0], cT[0])
        nc.sync.dma_start(xT_sb[:, 0], xT[0])
        nc.sync.dma_start(wv_sb[:], wv[:])
        for c in range(1, NCHUNK):
            nc.sync.dma_start(cT_sb[:, c], cT[c])
            nc.sync.dma_start(xT_sb[:, c], xT[c])
        nc.sync.dma_start(wo_sb[:], wo[:])

        # tiny dummy exp so the ~2.7us ACT_TABLE_LOAD overlaps the DMA wait
        nc.vector.memset(tbl[:, 0:1], 0.0)
        nc.scalar.activation(tbl[:, 1:2], tbl[:, 0:1], EXP)
        # warm-up matmuls on a small dedicated tile (NOT the big ones-memset
        # below, which takes ~4us on DVE and would delay them): the PE HAM
        # clock gate needs ~3.4us of sustained activity to lift the PE from
        # 1.2 to 2.4 GHz; burn that during the DMA wait so the first
        # projection chain runs at full clock.
        warm = const.tile([P, 512], BF16)
        nc.vector.memset(warm[:], 1.0)
        wps = ps_p.tile([P, 512], F32, tag="psp", name="warm")
        for _ in range(12):
            nc.tensor.matmul(
                wps[0:DH, :512],
                lhsT=warm[:, 0:DH],
                rhs=warm[:],
                start=True,
                stop=True,
            )
        # ones columns for the denominator trick; DVE is idle during the DMA
        nc.vector.memset(Vp_sb[:, :, :, DH:P], 1.0)

        # ---- just-in-time projections (emitted inside the attention loop
        # so PE reaches the first softmax chunk as soon as its DMA chunk
        # lands instead of draining all projection matmuls first) ----
        proj_done = set()

        def emit_qk(w_sb, src_sb, dst_sb, dt, ch, key):
            if (key, dt, ch) in proj_done:
                return
            proj_done.add((key, dt, ch))
            ps = ps_p.tile([P, 512], F32, tag="psp", name=f"{key}{dt}{ch}")
            for kt in range(KT_D):
                nc.tensor.matmul(
                    ps[:, :512],
                    lhsT=w_sb[:, kt, dt * P:(dt + 1) * P],
                    rhs=src_sb[:, ch, kt, :],
                    start=(kt == 0),
                    stop=(kt == KT_D - 1),
                )
            nc.vector.tensor_copy(
                dst_sb[:, dt, ch * 512:(ch + 1) * 512], ps[:, :512]
            )

        def emit_v(mt):
            if ("v", mt) in proj_done:
                return
            proj_done.add(("v", mt))
            ps = ps_p.tile([P, 512], F32, tag="psp", name=f"v{mt}")
            for kt in range(KT_D):
                nc.tensor.matmul(
                    ps[:, :DS],
                    lhsT=cT_sb[:, mt // 4, kt, (mt % 4) * P:(mt % 4 + 1) * P],
                    rhs=wv_sb[:, kt, :],
                    start=(kt == 0),
                    stop=(kt == KT_D - 1),
                )
            nc.vector.tensor_copy(
                Vp_sb[:, mt, :, 0:DH],
                ps[:, 0:DS].rearrange("p (h d) -> p h d", h=HPC),
            )

        # ---- Y^T = Wo_s^T O^T for one query chunk; emitted piecewise as
        # filler work inside the NEXT chunk's inner loop so it fills PE
        # slack during the ScalarE-bound attention instead of blocking the
        # in-order PE queue at chunk boundaries.
        NS = 512  # n sub-chunk
        ytiles = {}

        def emit_y(nch, dt4):
            n0 = nch * NS
            if dt4 == 0:
                ytiles[nch] = yout.tile(
                    [P, D // P, NS], BF16, tag="y", name=f"yt{nch}"
                )
            yt = ytiles[nch]
            ps = ps_p.tile([P, NS], F32, tag="psp", name=f"y{nch}{dt4}")
            for kt in range(DS // P):
                nc.tensor.matmul(
                    ps[:, :NS],
                    lhsT=wo_sb[:, kt, dt4 * P:(dt4 + 1) * P],
                    rhs=Ocat[:, kt, n0:n0 + NS],
                    start=(kt == 0),
                    stop=(kt == DS // P - 1),
                )
            nc.vector.tensor_copy(yt[:, dt4, :], ps[:, :NS])
            if dt4 == D // P - 1:
                nc.sync.dma_start(yT[nch], yt[:])

        # head pairs (2p, 2p+1) live at partition offsets 0/64 of d-tile p:
        # their K=64 S-matmuls use disjoint PE row groups (concurrent), and
        # share one [128, 1024] PSUM tile -> a single 1024-wide exp.
        # One flat software pipeline over all 8 (nch, pr) blocks x 16 m-tiles:
        # AV lags S/exp by LAG stages GLOBALLY (across block boundaries), so
        # a stalled AV (waiting on a po PSUM slot freed by the previous
        # block's normalize on DVE) never blocks the S matmuls that feed
        # ScalarE, and ScalarE enters each new block with a full runway.
        # Variable lag: the first two AV stages of each block wait on a po
        # PSUM slot that is freed only by the PREVIOUS block's normalize
        # chain on DVE (~2-4us after its last AV), so they lag 6 stages;
        # the rest lag 4.
        LAG = 4
        LAG0 = 6

        def avlag(ma):
            return LAG0 if ma < 2 else LAG

        NB = (N // NS) * (HPC // 2)   # 8 blocks
        TOT = NB * MT                 # 128 stages
        ebufs = [None] * (LAG0 + 2)
        po_blk = {}
        # ScalarE exp is the bottleneck engine (~141us busy); offload these
        # (block, mt) tiles to VectorE via the Schraudolph trick in bf16
        # space: exp(x) ~= bitcast_bf16(int16(x * 2^7/ln2 + (127*2^7 - C)))
        # one fused mult+add tensor_scalar with int16 convert-on-write, then
        # the AV matmul streams the tile as bf16 like the ACT path. ~1.7%
        # rms weight error on the offloaded key tiles; final output error
        # scales as 1.7% * sqrt(fraction offloaded).
        SCHR = {(g, mt) for g in range(2, NB) for mt in (6, 11)}
        EXA = SCALE * (1 << 7) / float(np.log(2.0))
        EXB = 127.0 * (1 << 7) - 366393.0 / (1 << 16)

        def fillers(nch, pr, mt):
            # background work with already-satisfied deps, spread over the
            # loop: pr-1 projections during block 0, the previous chunk's
            # output projection, the next chunk's Q projection
            if nch == 0 and pr == 0:
                if mt in (5, 7, 9, 11):
                    emit_qk(wk_sb, cT_sb, KT_sb, 1, (mt - 5) // 2, "k")
                if mt == 13:
                    emit_qk(wq_sb, xT_sb, QT_sb, 1, 0, "q")
            if pr == 0 and nch > 0 and mt in (4, 7, 10, 13):
                emit_y(nch - 1, (mt - 4) // 3)
            if pr == 1 and nch + 1 < N // NS and mt in (5, 9):
                emit_qk(wq_sb, xT_sb, QT_sb, (mt - 5) // 4, nch + 1, "q")

        from collections import deque
        pend = deque()
        for s in range(TOT + LAG0 + 1):
            if s < TOT:
                g, mt = divmod(s, MT)
                nch, pr = divmod(g, HPC // 2)
                n0 = nch * NS
                if mt == 0:
                    po_blk[g] = [
                        ps_o.tile([P, NS], F32, tag="po", name=f"po{g}{i}")
                        for i in range(2)
                    ]
                emit_qk(wk_sb, cT_sb, KT_sb, pr, mt // 4, "k")
                emit_qk(wq_sb, xT_sb, QT_sb, pr, n0 // 512, "q")
                fillers(nch, pr, mt)
                st = ps_s.tile([P, NCH], F32, tag="ps")
                for i in range(2):
                    dp = i * DH
                    nc.tensor.matmul(
                        st[:, i * NS:(i + 1) * NS],
                        lhsT=KT_sb[dp:dp + DH, pr, mt * P:(mt + 1) * P],
                        rhs=QT_sb[dp:dp + DH, pr, n0:n0 + NS],
                        start=True,
                        stop=True,
                    )
                if (g, mt) in SCHR:
                    ei = work.tile([P, NCH], mybir.dt.int16, tag="ei",
                                   name=f"ei{g}")
                    nc.vector.tensor_scalar(
                        ei[:], st[:], EXA, EXB,
                        mybir.AluOpType.mult, mybir.AluOpType.add,
                    )
                    ebufs[s % len(ebufs)] = ei
                else:
                    e = work.tile([P, NCH], BF16, tag="e")
                    nc.scalar.activation(e[:], st[:], EXP, scale=SCALE)
                    ebufs[s % len(ebufs)] = e
                pend.append(s)
            # ascending order keeps start=True (ma==0) first in the PE queue
            while pend and pend[0] + avlag(pend[0] % MT) <= s:
                sa = pend.popleft()
                g, ma = divmod(sa, MT)
                nch, pr = divmod(g, HPC // 2)
                n0 = nch * NS
                emit_v(ma)
                e = ebufs[sa % len(ebufs)]
                schr = (g, ma) in SCHR
                for i, h in enumerate((2 * pr, 2 * pr + 1)):
                    rhs = e[:, i * NS:(i + 1) * NS]
                    if schr:
                        rhs = rhs.bitcast(BF16)
                    nc.tensor.matmul(
                        po_blk[g][i][:],
                        lhsT=Vp_sb[:, ma, h, :],
                        rhs=rhs,
                        start=(ma == 0),
                        stop=(ma == MT - 1),
                    )
                if ma == MT - 1:
                    # normalize: O^T = O'^T * (1/sums); sums already sit on
                    # rows 64..127 of the po accumulators
                    for i in range(2):
                        dp = i * DH
                        sc = work.tile([DH, NS], F32, tag="sc")
                        nc.vector.tensor_copy(sc[:], po_blk[g][i][DH:P, :])
                        rc = work.tile([DH, NS], F32, tag="rc")
                        nc.vector.reciprocal_approx_fast(rc[:], sc[:])
                        nc.vector.tensor_tensor(
                            Ocat[dp:dp + DH, pr, n0:n0 + NS],
                            po_blk[g][i][0:DH, :],
                            rc[:],
                            mybir.AluOpType.mult,
                        )
                    del po_blk[g]

        # last chunk's output projection runs in the tail
        for dt4 in range(D // P):
            emit_y(N // NS - 1, dt4)


def _install_ntff_hook():
    """Best-effort NTFF profiling under axon: provide the antenv.axon_hooks
    shim the boot code looks for, and avoid the artifact upload."""
    try:
        import sys
        import types

        import concourse.bass_utils as bu

        bu.upload_artifacts = lambda d: d  # no S3 in this sandbox
        try:
            from antenv.axon_hooks import get_axon_ntff_profile_hook  # noqa: F401
            return  # already present
        except ImportError:
            pass
        import antenv
        from trn_agent_boot.trn_boot import _ntff_profile_via_ctypes

        mod = types.ModuleType("antenv.axon_hooks")
        _state = {"hook": _ntff_profile_via_ctypes("/opt/axon/libaxon_pjrt.so")}
        mod.set_axon_ntff_profile_hook = lambda h: _state.__setitem__("hook", h)
        mod.get_axon_ntff_profile_hook = lambda: _state["hook"]
        sys.modules["antenv.axon_hooks"] = mod
        antenv.axon_hooks = mod
    except Exception as e:  # pragma: no cover
        print(f"ntff hook install failed ({e}); running without trace")


def _pack_act(a):
    """[D, N] -> [chunk, p, ko, 512] so each chunk DMA is one contiguous
    4KB descriptor per partition."""
    return np.ascontiguousarray(
        a.reshape(KT_D, P, NCHUNK, CW).transpose(2, 1, 0, 3)
    ).astype(_NBF)


def _pack_w(w):
    """[D, DS] -> [p, ko, d]."""
    ds = w.shape[1]
    return np.ascontiguousarray(
        w.reshape(KT_D, P, ds).transpose(1, 0, 2)
    ).astype(_NBF)


def kernel(x, context, Wq, Wk, Wv, Wo, bo):
    x = np.asarray(x, dtype=np.float32)
    context = np.asarray(context, dtype=np.float32)
    Wq = np.asarray(Wq, dtype=np.float32)
    Wk = np.asarray(Wk, dtype=np.float32)
    Wv = np.asarray(Wv, dtype=np.float32)
    Wo = np.asarray(Wo, dtype=np.float32)
    bo = np.asarray(bo, dtype=np.float32)
    B = x.shape[0]

    in_maps = []
    for c in range(8):
        b, hg = c // 2, c % 2
        sl = slice(hg * DS, (hg + 1) * DS)
        wos = Wo[sl, :]  # [DS, D]
        in_maps.append({
            "xT": _pack_act(x[b].T),
            "cT": _pack_act(context[b].T),
            "wq": _pack_w(Wq[:, sl]),
            "wk": _pack_w(Wk[:, sl]),
            "wv": _pack_w(Wv[:, sl]),
            "wo": np.ascontiguousarray(
                wos.reshape(DS // P, P, D).transpose(1, 0, 2)
            ).astype(_NBF),
        })

    nc = _build_nc()
    trace = bool(int(os.environ.get("BASS_KERNEL_TRACE", "0")))
    if trace:
        _install_ntff_hook()
    res = run_bass_kernel_spmd(nc, in_maps, list(range(8)), trace=trace)
    if trace and res.exec_time_ns is not None:
        print(f"HW exec time: {res.exec_time_ns} ns")

    out = np.empty((B, N, D), dtype=np.float32)
    for b in range(B):
        # yT is [nch, p, dt, n'] bf16; d = dt*128 + p, n = nch*512 + n'
        y0 = res.results[2 * b]["yT"].astype(np.float32)
        y1 = res.results[2 * b + 1]["yT"].astype(np.float32)
        yt = (y0 + y1).transpose(2, 1, 0, 3).reshape(D, N)
        out[b] = yt.T + bo[None, :]
    return out
